# revision 8
# baseline (speedup 1.0000x reference)
"""GroupedQueryAttention, tensor-parallel over heads on 8 NeuronCores (raw Bass).

Core c owns q heads {2c, 2c+1} and kv head c//2. All matmul operands bf16
(f32 PSUM), inputs host-cast/transposed/pre-arranged for contiguous DMA.
Device pipeline per core:
  load hsT (host-transposed, chunk-contiguous) -> qkv projections (PE,
  q 2-head-packed, k duplicated into both partition halves) -> RoPE (DVE;
  swap-halves DMAs: q on ACT queue, k on gpsimd queue; sign folded into
  host ssin table) ->
  attention in 4 query-quarters of 512, both heads per step:
    per kt: S^T_h = kT2[h].T @ qT2[h] as two CONCURRENT 64-row PE tiles
    (h0 rows 0-63, h1 rows 64-127) into adjacent PSUM banks ->
    one exp FD=1024 covers both heads (ACT, bf16 out, PT ring of 4) ->
    PV per head accumulates [65,512] (fused denominator row) over 16 kt ->
    raw-copy pv -> SBUF, den -> [64,16], recip, scr2 DRAM bounce,
    stride-0 broadcast rb, normalize -> attn_sb[:, quarter] ->
    AllGather [128,512] per quarter (hides under later quarters) ->
  o_proj per quarter (8-MM accumulation) gated on the NEXT collective
  completing (per-sender SDMA FIFO proof), tiny barrier gather at end.
Host: out[:, c*128:(c+1)*128] = out_t_c.T.
"""
import sys, os
sys.path.insert(0, '/opt/trn_rl_repo')
import contextlib
import numpy as np
import ml_dtypes
import concourse.bass as bass
import concourse.mybir as mybir
from concourse.bass_utils import run_bass_kernel_spmd

F32 = mybir.dt.float32
BF16 = mybir.dt.bfloat16
EXP = mybir.ActivationFunctionType.Exp
NPBF = ml_dtypes.bfloat16

S, HID, HD = 2048, 1024, 64
NCORES = 8
NST = S // 128      # 16 k tiles
NHT = HID // 128    # 8 contraction tiles
NSC = 4             # 512-wide seq chunks (front phase + attention quarters)
NPT = 4             # PT ring size
NG = NSC * NST      # 64 global attention steps


def build_kernel():
    nc = bass.Bass("TRN2", target_bir_lowering=False, num_devices=NCORES)

    hsT_d = nc.dram_tensor("hst", [NSC, 128, NHT * 512], BF16, kind="ExternalInput")
    wq_d = nc.dram_tensor("wq", [128, NHT * 128], BF16, kind="ExternalInput")
    wkk_d = nc.dram_tensor("wkk", [128, NHT * 128], BF16, kind="ExternalInput")
    wv_d = nc.dram_tensor("wv", [128, NHT * HD], BF16, kind="ExternalInput")
    wo_d = nc.dram_tensor("wo", [128, NHT * 128], BF16, kind="ExternalInput")
    cosT_d = nc.dram_tensor("cost", [128, S], BF16, kind="ExternalInput")
    ssinT_d = nc.dram_tensor("ssint", [128, S], BF16, kind="ExternalInput")
    id_d = nc.dram_tensor("ident", [128, 128], BF16, kind="ExternalInput")
    out_d = nc.dram_tensor("out_t", [128, S], F32, kind="ExternalOutput")
    scr2_d = nc.dram_tensor("scr2", [NSC, 1024], F32)
    agin_d = nc.dram_tensor("agin", [NSC, 128, 512], BF16)
    agout_d = nc.dram_tensor("agout", [NSC, NCORES, 128, 512], BF16, addr_space="Shared")
    bar_d = nc.dram_tensor("bar", [128], F32)
    barout_d = nc.dram_tensor("barout", [NCORES, 128], F32, addr_space="Shared")

    def sb(name, shape, dt):
        return nc.alloc_sbuf_tensor(name, shape, dt).ap()

    hsT = sb("hsT", [128, NHT, S], BF16)
    ident = sb("ident_sb", [128, 128], BF16)
    cosT2 = sb("cosT2", [128, S], BF16)
    ssinT2 = sb("ssinT2", [128, S], BF16)
    wq_sb = sb("wq_sb", [128, NHT, 128], BF16)
    wkk_sb = sb("wkk_sb", [128, NHT, 128], BF16)
    wv_sb = sb("wv_sb", [128, NHT, HD], BF16)
    wo_sb = sb("wo_sb", [128, NHT, 128], BF16)
    q2 = [sb(f"q2_{i}", [128, 512], BF16) for i in range(2)]
    qs2 = [sb(f"qs2_{i}", [128, 512], BF16) for i in range(2)]
    k2 = [sb(f"k2_{i}", [128, 512], BF16) for i in range(2)]
    ks2 = [sb(f"ks2_{i}", [128, 512], BF16) for i in range(2)]
    tmpa = sb("tmpa", [128, 512], BF16)
    tmpb = sb("tmpb", [128, 512], BF16)
    qT2 = sb("qT2", [128, S], BF16)
    kT2 = sb("kT2", [128, S], BF16)
    vT = sb("vT", [HD, S], BF16)
    vaug = sb("vaug", [128, NST, HD + 1], BF16)
    PT = [sb(f"PT{i}", [128, 1024], BF16) for i in range(NPT)]
    araw = [sb(f"araw{h}", [HD + 1, S], BF16) for h in range(2)]
    den64 = sb("den64", [64, 16], BF16)
    rcp64 = sb("rcp64", [64, 16], F32)
    rb = [[sb(f"rb{p}_{h}", [HD, 512], F32) for h in range(2)] for p in range(2)]
    attn_sb = sb("attn_sb", [128, S], BF16)
    af = [sb(f"af{q}", [128, NHT, 512], BF16) for q in range(NSC)]
    out_ch = [sb(f"out_ch{i}", [128, 512], F32) for i in range(2)]
    dum_sb = sb("dum_sb", [1, 16], F32)
    dumb_sb = sb("dumb_sb", [1, 16], BF16)

    ps = nc.alloc_psum_tensor("psblob", [128, 4096], F32).ap()
    # front phase
    qp = [ps[:, 0:512], ps[:, 512:1024]]
    kp = [ps[:, 1024:1536], ps[:, 1536:2048]]
    vp = [ps[0:HD, 2048:2560], ps[0:HD, 2560:3072]]
    vtp = ps[:, 3072:3584].bitcast(BF16)          # [128, 1024]: 16 x [128,64]
    # attention: sp pair (g%2): banks (0,1) even g, (2,3) odd g
    spx = [ps[:, 0:1024], ps[:, 1024:2048]]       # exp reads FD=1024
    pv = [ps[0:HD + 1, 2048:2560], ps[0:HD + 1, 2560:3072]]  # per-head [65,512]
    opb = [ps[:, 3072:3584], ps[:, 3584:4096]]    # o_proj accum (parity)

    es = contextlib.ExitStack()
    SEM = lambda n: es.enter_context(nc.semaphore(n))
    sL = SEM("sL")        # SP weight/table loads (+16)
    sHSa = SEM("sHSa")    # hsT chunks 0,2 (SP)
    sHSb = SEM("sHSb")    # hsT chunks 1,3 (gpsimd)
    sQP = SEM("sQP"); sKP = SEM("sKP"); sVP = SEM("sVP")
    sQC = SEM("sQC"); sKC = SEM("sKC"); sVC = SEM("sVC")
    sSWQ = SEM("sSWQ")    # q swap DMAs (+16 each, 4/chunk, ACT queue)
    sSWK = SEM("sSWK")    # k swap DMAs (+16 each, 4/chunk, gpsimd queue)
    sQR = SEM("sQR"); sKR = SEM("sKR")
    sVA = SEM("sVA"); sMS = SEM("sMS")
    sSC = SEM("sSC")      # score MMs (2/g)
    sEX = SEM("sEX")      # exps (1/g)
    sPV = SEM("sPV")      # PV steps (1/g, on pv1 MM)
    sPVD = SEM("sPVD")    # pv0 stop MMs (1/quarter)
    sVTD = SEM("sVTD")    # PE drain after each chunk's v transposes
    sRW = SEM("sRW")      # raw attn copies out of psum (2/quarter)
    sDN64 = SEM("sDN64")  # den64 DMAs (+16 each, 3/quarter)
    sRC = SEM("sRC")      # recips (1/quarter)
    sDNS = SEM("sDNS")    # scr2 writes (+16 each, 2/quarter)
    sRB = SEM("sRB")      # rb broadcasts (+16 each, 3/quarter)
    sNM = SEM("sNM")      # normalize muls (2/quarter)
    sAG = SEM("sAG")      # agin DMAs (+16 each, 2/quarter)
    sCC = SEM("sCC")      # collectives (4 AG + barrier)
    sAF = SEM("sAF")      # af loads on SP queue (+16 each, 1/quarter + dummy)
    sOP = SEM("sOP")      # o_proj stop MMs (1/quarter)
    sOC = SEM("sOC"); sOD = SEM("sOD")

    with nc.Block() as block:

        # ===== SP: loads + den chain + af + stores =====
        @block.sync
        def _(sync):
            sync.dma_start(out=wkk_sb, in_=wkk_d[:]).then_inc(sL, 16)
            sync.dma_start(out=hsT[:, :, 0:512], in_=hsT_d[0]).then_inc(sHSa, 16)
            sync.dma_start(out=wq_sb, in_=wq_d[:]).then_inc(sL, 16)
            sync.dma_start(out=wv_sb, in_=wv_d[:]).then_inc(sL, 16)
            sync.dma_start(out=hsT[:, :, 1024:1536], in_=hsT_d[2]).then_inc(sHSa, 16)
            sync.dma_start(out=ident, in_=id_d[:]).then_inc(sL, 16)
            sync.dma_start(out=cosT2, in_=cosT_d[:]).then_inc(sL, 16)
            sync.dma_start(out=ssinT2, in_=ssinT_d[:]).then_inc(sL, 16)
            sync.dma_start(out=wo_sb, in_=wo_d[:]).then_inc(sL, 16)

            def den_agin(qc):
                sl = slice(qc * 512, (qc + 1) * 512)
                # rcp -> DRAM; read-back completion proves DRAM visibility
                # for the gpsimd queue's broadcast reads
                sync.wait_ge(sRC, qc + 1)
                sync.dma_start(out=scr2_d[qc], in_=rcp64).then_inc(sDNS, 16)
                sync.dma_start(out=dum_sb, in_=scr2_d[qc, 0:16]).then_inc(sDNS, 16)
                sync.wait_ge(sNM, 2 * qc + 2)
                sync.dma_start(out=agin_d[qc], in_=attn_sb[:, sl]).then_inc(sAG, 16)
                sync.dma_start(out=dumb_sb, in_=agin_d[qc][0:1, 0:16]).then_inc(sAG, 16)

            def af_load(qc):
                # A collective's completion sem does NOT order remote ranks'
                # inbound pushes against our reads. Per-sender SDMA queues are
                # FIFO, so entering the NEXT collective proves every rank's
                # previous pushes landed: gate af[qc] on collective qc+1
                # (the trailing barrier gather for qc=3).
                sync.wait_ge(sCC, qc + 2)
                # af[p, j, q] = agout[core j, row p, q]; slab j of Wo is rows
                # j*128..j*128+127 (natural order: row (2c+h)*64+d = c*128+p)
                sync.dma_start(
                    out=af[qc],
                    in_=agout_d[qc].rearrange("c p q -> p c q"),
                ).then_inc(sAF, 16)

            def store(qc):
                sync.wait_ge(sOC, qc + 1)
                sync.dma_start(
                    out=out_d[:, qc * 512:(qc + 1) * 512], in_=out_ch[qc % 2]
                ).then_inc(sOD, 16)

            den_agin(0)
            den_agin(1)
            af_load(0)
            den_agin(2)
            af_load(1)
            den_agin(3)
            af_load(2)
            # +1-shift proof DMAs: a later DMA on this FIFO queue proves the
            # preceding af load's SBUF writes are visible to the PE
            sync.dma_start(out=dum_sb, in_=scr2_d[0, 0:16]).then_inc(sAF, 16)
            store(0)
            af_load(3)
            sync.dma_start(out=dum_sb, in_=scr2_d[0, 0:16]).then_inc(sAF, 16)
            store(1)
            store(2)
            store(3)
            sync.wait_ge(sOD, 16 * NSC)

        # ================= PE =================
        @block.tensor
        def _(tensor):
            for sc in range(NSC):
                if sc % 2 == 0:
                    tensor.wait_ge(sHSa, 16 * (sc // 2 + 1))
                else:
                    tensor.wait_ge(sHSb, 16 * (sc // 2 + 1))
                sl = slice(sc * 512, (sc + 1) * 512)
                # k projection (duplicated into both halves)
                if sc == 0:
                    tensor.wait_ge(sL, 16)
                if sc >= 2:
                    tensor.wait_ge(sKC, sc - 1)
                for ht in range(NHT):
                    inst = tensor.matmul(
                        kp[sc % 2], wkk_sb[:, ht, :], hsT[:, ht, sl],
                        start=(ht == 0), stop=(ht == NHT - 1),
                    )
                tensor.drain().then_inc(sKP, 1)
                # q projection (2 heads packed)
                if sc == 0:
                    tensor.wait_ge(sL, 32)
                if sc >= 2:
                    tensor.wait_ge(sQC, sc - 1)
                for ht in range(NHT):
                    inst = tensor.matmul(
                        qp[sc % 2], wq_sb[:, ht, :], hsT[:, ht, sl],
                        start=(ht == 0), stop=(ht == NHT - 1),
                    )
                tensor.drain().then_inc(sQP, 1)
                # v projection
                if sc == 0:
                    tensor.wait_ge(sL, 48)
                if sc >= 2:
                    tensor.wait_ge(sVC, sc - 1)
                for ht in range(NHT):
                    inst = tensor.matmul(
                        vp[sc % 2], wv_sb[:, ht, :], hsT[:, ht, sl],
                        start=(ht == 0), stop=(ht == NHT - 1),
                    )
                tensor.drain().then_inc(sVP, 1)
                # v transposes for this chunk's 4 ktiles
                if sc == 0:
                    tensor.wait_ge(sL, 64)
                tensor.wait_ge(sVC, sc + 1)
                for j in range(4):
                    kt = 4 * sc + j
                    tensor.transpose(
                        vtp[:, kt * 64:(kt + 1) * 64],
                        vT[:, kt * 128:(kt + 1) * 128],
                        ident[0:HD, 0:HD],
                    )
                tensor.drain().then_inc(sVTD, 1)

            # ================= attention (4 query quarters) =============
            tensor.wait_ge(sQC, NSC)
            tensor.wait_ge(sKC, NSC)
            tensor.wait_ge(sVC, NSC)
            tensor.wait_ge(sMS, 1)

            def pv_step(gp):
                qcp, ktp = gp // NST, gp % NST
                tensor.wait_ge(sEX, gp + 1)
                tensor.wait_ge(sVA, ktp + 1)
                if ktp == 0 and qcp > 0:
                    tensor.wait_ge(sRW, 2 * qcp)
                st, sp_ = (ktp == 0), (ktp == NST - 1)
                i0 = tensor.matmul(
                    pv[0], vaug[:, ktp, :], PT[gp % NPT][:, 0:512],
                    start=st, stop=sp_,
                )
                i1 = tensor.matmul(
                    pv[1], vaug[:, ktp, :], PT[gp % NPT][:, 512:1024],
                    start=st, stop=sp_,
                )
                i1.then_inc(sPV, 1)
                if sp_:
                    i0.then_inc(sPVD, 1)

            for g in range(NG):
                qc, kt = g // NST, g % NST
                if g >= 2:
                    tensor.wait_ge(sEX, g - 1)   # sp pair free
                if kt == 0:
                    tensor.wait_ge(sQR, qc + 1)
                if qc == 0:
                    tensor.wait_ge(sKR, kt // 4 + 1)
                qsl = slice(qc * 512, (qc + 1) * 512)
                for h in range(2):
                    hp = slice(h * 64, (h + 1) * 64)
                    tensor.matmul(
                        ps[:, (g % 2) * 1024 + h * 512:(g % 2) * 1024 + (h + 1) * 512],
                        kT2[hp, kt * 128:(kt + 1) * 128],
                        qT2[hp, qsl],
                        start=True, stop=True,
                    ).then_inc(sSC, 1)
                if g >= 1:
                    pv_step(g - 1)
            pv_step(NG - 1)

            # ================= o_proj (per quarter, 8-MM accum) =========
            tensor.wait_ge(sL, 112)
            # af gating +1-shifted; sAF increments: af0=16, af1=32, af2=48,
            # dummy=64, af3=80, dummy=96
            for qc, afw in enumerate((32, 48, 64, 96)):
                tensor.wait_ge(sAF, afw)
                if qc >= 2:
                    tensor.wait_ge(sOC, qc - 1)
                for j in range(NHT):
                    inst = tensor.matmul(
                        opb[qc % 2], wo_sb[:, j, :], af[qc][:, j, :],
                        start=(j == 0), stop=(j == NHT - 1),
                    )
                inst.then_inc(sOP, 1)

        # ================= DVE =================
        @block.vector
        def _(vector):
            for sc in range(NSC):
                sl = slice(sc * 512, (sc + 1) * 512)
                vector.wait_ge(sQP, sc + 1)
                if sc >= 2:
                    vector.wait_ge(sSWQ, 64 * (sc - 1))  # q2 swap reads done
                vector.tensor_copy(q2[sc % 2], qp[sc % 2]).then_inc(sQC, 1)
                vector.wait_ge(sKP, sc + 1)
                if sc >= 2:
                    vector.wait_ge(sSWK, 64 * (sc - 1))  # k2 swap reads done
                vector.tensor_copy(k2[sc % 2], kp[sc % 2]).then_inc(sKC, 1)
                vector.wait_ge(sVP, sc + 1)
                vector.tensor_copy(vT[:, sl], vp[sc % 2]).then_inc(sVC, 1)
                # rope q (both heads packed)
                if sc == 0:
                    vector.wait_ge(sL, 96)
                vector.wait_ge(sSWQ, 64 * (sc + 1))
                vector.tensor_mul(tmpa, q2[sc % 2], cosT2[:, sl])
                vector.tensor_mul(tmpb, qs2[sc % 2], ssinT2[:, sl])
                vector.tensor_add(qT2[:, sl], tmpa, tmpb).then_inc(sQR, 1)
                # rope k
                vector.wait_ge(sSWK, 64 * (sc + 1))
                vector.tensor_mul(tmpa, k2[sc % 2], cosT2[:, sl])
                vector.tensor_mul(tmpb, ks2[sc % 2], ssinT2[:, sl])
                vector.tensor_add(kT2[:, sl], tmpa, tmpb).then_inc(sKR, 1)
                # vaug copies
                vector.wait_ge(sVTD, sc + 1)
                for j in range(4):
                    kt = 4 * sc + j
                    vector.tensor_copy(vaug[:, kt, 0:HD], vtp[:, kt * 64:(kt + 1) * 64]).then_inc(sVA, 1)

            # per-quarter: raw copies (release pv psum), recip, normalize
            for qc in range(NSC):
                sl = slice(qc * 512, (qc + 1) * 512)
                vector.wait_ge(sPVD, qc + 1)
                vector.wait_ge(sPV, NST * (qc + 1))
                vector.tensor_copy(araw[0][:, sl], pv[0]).then_inc(sRW, 1)
                vector.tensor_copy(araw[1][:, sl], pv[1]).then_inc(sRW, 1)
                vector.wait_ge(sDN64, 48 * (qc + 1))
                if qc >= 1:
                    vector.wait_ge(sDNS, 32 * qc - 16)  # rcp64 drained (qc-1)
                vector.reciprocal(rcp64, den64).then_inc(sRC, 1)
                # +1-DMA shift: sw-DGE completion sems can fire before the
                # data is visible; the NEXT DMA's completion (same queue,
                # FIFO) proves this one's writes landed
                vector.wait_ge(sRB, 48 * (qc + 1))
                vector.tensor_mul(attn_sb[0:64, sl], araw[0][0:HD, sl], rb[qc % 2][0]).then_inc(sNM, 1)
                vector.tensor_mul(attn_sb[64:128, sl], araw[1][0:HD, sl], rb[qc % 2][1]).then_inc(sNM, 1)
            # out copies (after the last quarter's den chain so the AG3
            # critical path is not blocked behind o_proj results)
            for oq in range(NSC):
                vector.wait_ge(sOP, oq + 1)
                if oq >= 2:
                    vector.wait_ge(sOD, 16 * (oq - 1))
                vector.tensor_copy(out_ch[oq % 2], opb[oq % 2]).then_inc(sOC, 1)

        # ====== ACT: hsT ch1/ch3 loads + swap DMAs + exp (hw DGE) ======
        @block.scalar
        def _(scalar):
            scalar.dma_start(out=hsT[:, :, 512:1024], in_=hsT_d[1]).then_inc(sHSb, 16)
            scalar.dma_start(out=hsT[:, :, 1536:2048], in_=hsT_d[3]).then_inc(sHSb, 16)
            for sc in range(NSC):
                scalar.wait_ge(sQC, sc + 1)
                if sc >= 2:
                    scalar.wait_ge(sQR, sc - 1)  # qs2 buf consumed
                for b in range(2):
                    scalar.dma_start(
                        out=qs2[sc % 2][b * 64:b * 64 + 32, :],
                        in_=q2[sc % 2][b * 64 + 32:b * 64 + 64, :],
                    ).then_inc(sSWQ, 16)
                    scalar.dma_start(
                        out=qs2[sc % 2][b * 64 + 32:b * 64 + 64, :],
                        in_=q2[sc % 2][b * 64:b * 64 + 32, :],
                    ).then_inc(sSWQ, 16)
                scalar.wait_ge(sKC, sc + 1)
                if sc >= 2:
                    scalar.wait_ge(sKR, sc - 1)  # ks2 buf consumed
                for b in range(2):
                    scalar.dma_start(
                        out=ks2[sc % 2][b * 64:b * 64 + 32, :],
                        in_=k2[sc % 2][b * 64 + 32:b * 64 + 64, :],
                    ).then_inc(sSWK, 16)
                    scalar.dma_start(
                        out=ks2[sc % 2][b * 64 + 32:b * 64 + 64, :],
                        in_=k2[sc % 2][b * 64:b * 64 + 32, :],
                    ).then_inc(sSWK, 16)
            # exps: one per global step covers both heads (FD=1024)
            for g in range(NG):
                scalar.wait_ge(sSC, 2 * g + 2)
                if g >= NPT:
                    scalar.wait_ge(sPV, g - (NPT - 1))  # PT slot consumed
                scalar.activation(
                    PT[g % NPT][:, :], spx[g % 2], EXP, scale=0.125,
                ).then_inc(sEX, 1)

        # ==== GPSIMD: memset, sw-DGE den/broadcast DMAs, collectives ====
        @block.gpsimd
        def _(gpsimd):
            gpsimd.memset(vaug[:, :, HD:HD + 1], 1.0).then_inc(sMS, 1)
            for qc in range(NSC):
                sl = slice(qc * 512, (qc + 1) * 512)
                gpsimd.wait_ge(sRW, 2 * qc + 2)
                if qc >= 1:
                    gpsimd.wait_ge(sRC, qc)  # den64 consumed by recip qc-1
                gpsimd.dma_start(
                    out=den64[0:32, :], in_=araw[0][HD:HD + 1, sl],
                ).then_inc(sDN64, 16)
                gpsimd.dma_start(
                    out=den64[32:64, :], in_=araw[1][HD:HD + 1, sl],
                ).then_inc(sDN64, 16)
                # dummy follow-up DMA: its completion proves den64 is visible
                # (sw-DGE completion sems can fire before the data lands)
                gpsimd.dma_start(
                    out=dumb_sb, in_=araw[0][HD:HD + 1, qc * 512:qc * 512 + 16],
                ).then_inc(sDN64, 16)
                gpsimd.wait_ge(sDNS, 32 * (qc + 1))
                if qc >= 2:
                    gpsimd.wait_ge(sNM, 2 * qc - 2)  # rb parity buf consumed
                for h in range(2):
                    gpsimd.dma_start(
                        out=rb[qc % 2][h],
                        in_=bass.AP(scr2_d[:].tensor, qc * 1024 + h * 512, [[0, HD], [1, 512]]),
                    ).then_inc(sRB, 16)
                gpsimd.dma_start(
                    out=dum_sb, in_=bass.AP(scr2_d[:].tensor, qc * 1024, [[0, 1], [1, 16]]),
                ).then_inc(sRB, 16)
                gpsimd.wait_ge(sAG, 32 * (qc + 1))
                gpsimd.collective_compute(
                    "AllGather",
                    mybir.AluOpType.bypass,
                    replica_groups=[list(range(NCORES))],
                    ins=[agin_d[qc]],
                    outs=[agout_d[qc]],
                ).then_inc(sCC, 1)
            # barrier collective: its completion proves every rank's AG3
            # pushes into our agout landed (FIFO per sender SDMA queue)
            gpsimd.collective_compute(
                "AllGather",
                mybir.AluOpType.bypass,
                replica_groups=[list(range(NCORES))],
                ins=[bar_d[:]],
                outs=[barout_d[:]],
            ).then_inc(sCC, 1)

    es.close()
    return nc


_NC_CACHE = None


def kernel(hidden_states, cos, sin, attention_mask, Wq, Wk, Wv, Wo):
    global _NC_CACHE
    if _NC_CACHE is None:
        _NC_CACHE = build_kernel()
    nc = _NC_CACHE
    hs2 = np.asarray(hidden_states, dtype=np.float32).reshape(S, HID)
    # hsT chunk-contiguous: [sc, p, t*512] with row (t*128+p) of hs.T
    hsT = np.ascontiguousarray(hs2.T.astype(NPBF))                    # [HID, S]
    hsT_c = np.ascontiguousarray(
        hsT.reshape(NHT, 128, NSC, 512).transpose(2, 1, 0, 3).reshape(NSC, 128, NHT * 512))
    cosT = np.asarray(cos, dtype=np.float32).reshape(S, HD).T         # [64, S]
    sinT = np.asarray(sin, dtype=np.float32).reshape(S, HD).T
    ssinT = sinT.copy()
    ssinT[0:32, :] *= -1.0
    cosT2 = np.ascontiguousarray(np.concatenate([cosT, cosT], 0).astype(NPBF))
    ssinT2 = np.ascontiguousarray(np.concatenate([ssinT, ssinT], 0).astype(NPBF))
    Wq = np.asarray(Wq, dtype=np.float32)
    Wk = np.asarray(Wk, dtype=np.float32)
    Wv = np.asarray(Wv, dtype=np.float32)
    Wo = np.asarray(Wo, dtype=np.float32)
    ident = np.eye(128, dtype=np.float32).astype(NPBF)

    def warr(w):  # [1024, X] -> [128, 8*X] partition-major contiguous
        x = w.shape[1]
        return np.ascontiguousarray(
            w.reshape(NHT, 128, x).transpose(1, 0, 2).reshape(128, NHT * x).astype(NPBF))

    in_maps = []
    for c in range(NCORES):
        g = c // 2
        wk_g = Wk[:, g * HD:(g + 1) * HD]
        in_maps.append({
            "hst": hsT_c,
            "wq": warr(Wq[:, c * 128:(c + 1) * 128]),
            "wkk": warr(np.concatenate([wk_g, wk_g], axis=1)),
            "wv": warr(Wv[:, g * HD:(g + 1) * HD]),
            "wo": warr(Wo[:, c * 128:(c + 1) * 128]),
            "cost": cosT2,
            "ssint": ssinT2,
            "ident": ident,
        })
    res = run_bass_kernel_spmd(nc, in_maps, core_ids=list(range(NCORES)),
                               trace=bool(int(os.environ.get("KERNEL_TRACE", "0"))))
    out = np.empty((S, HID), dtype=np.float32)
    for c in range(NCORES):
        out[:, c * 128:(c + 1) * 128] = res.results[c]["out_t"].T
    kernel.last_results = res
    return out.reshape(1, S, HID)


if __name__ == "__main__":
    import tempfile
    from concourse.bass_utils import compile_bass_kernel
    nc = build_kernel()
    with tempfile.TemporaryDirectory() as td:
        compile_bass_kernel(nc, td)
    print("COMPILE OK")


# revision 27
# speedup vs baseline: 1.2771x; 1.2771x over previous
"""GroupedQueryAttention, tensor-parallel over heads on 8 NeuronCores (raw Bass).

Core c owns q heads {2c, 2c+1} and kv head c//2. All matmul operands bf16
(f32 PSUM), inputs host-cast/transposed/pre-arranged for contiguous DMA.
Device pipeline per core:
  load hsT (host-transposed, chunk-contiguous) -> qkv projections (PE,
  q 2-head-packed, k duplicated into both partition halves) -> RoPE (DVE;
  swap-halves DMAs: q on ACT queue, k on gpsimd queue; sign folded into
  host ssin table) ->
  attention in 4 query-quarters of 512, both heads per step:
    per kt: S^T_h = kT2[h].T @ qT2[h] as two CONCURRENT 64-row PE tiles
    (h0 rows 0-63, h1 rows 64-127) into adjacent PSUM banks ->
    one exp FD=1024 covers both heads (ACT, bf16 out, PT ring of 4) ->
    PV per head accumulates [65,512] (fused denominator row) over 16 kt ->
    raw-copy pv -> SBUF, den -> [64,16], recip, scr2 DRAM bounce,
    stride-0 broadcast rb, normalize -> attn_sb[:, quarter] ->
    AllGather [128,512] per quarter (hides under later quarters) ->
  o_proj per quarter (8-MM accumulation) gated on the NEXT collective
  completing (per-sender SDMA FIFO proof), tiny barrier gather at end.
Host: out[:, c*128:(c+1)*128] = out_t_c.T.
"""
import sys, os
sys.path.insert(0, '/opt/trn_rl_repo')
import contextlib
import numpy as np
import ml_dtypes
import concourse.bass as bass
import concourse.mybir as mybir
from concourse.bass_utils import run_bass_kernel_spmd

F32 = mybir.dt.float32
BF16 = mybir.dt.bfloat16
EXP = mybir.ActivationFunctionType.Exp
NPBF = ml_dtypes.bfloat16

S, HID, HD = 2048, 1024, 64
NCORES = 8
NST = S // 128      # 16 k tiles
NHT = HID // 128    # 8 contraction tiles
NSC = 4             # 512-wide seq chunks (front phase + attention quarters)
NPT = 4             # PT ring size
NG = NSC * NST      # 64 global attention steps


def build_kernel():
    nc = bass.Bass("TRN2", target_bir_lowering=False, num_devices=NCORES)

    hsT_d = nc.dram_tensor("hst", [NSC, 128, NHT * 512], BF16, kind="ExternalInput")
    wq_d = nc.dram_tensor("wq", [128, NHT * 128], BF16, kind="ExternalInput")
    wkk_d = nc.dram_tensor("wkk", [128, NHT * 128], BF16, kind="ExternalInput")
    wv_d = nc.dram_tensor("wv", [128, NHT * HD], BF16, kind="ExternalInput")
    wo_d = nc.dram_tensor("wo", [128, NHT * 128], BF16, kind="ExternalInput")
    cosT_d = nc.dram_tensor("cost", [128, S], BF16, kind="ExternalInput")
    ssinT_d = nc.dram_tensor("ssint", [128, S], BF16, kind="ExternalInput")
    id_d = nc.dram_tensor("ident", [128, 128], BF16, kind="ExternalInput")
    out_d = nc.dram_tensor("out_t", [128, S], F32, kind="ExternalOutput")
    scr2_d = nc.dram_tensor("scr2", [NSC, 1024], F32)
    agin_d = nc.dram_tensor("agin", [NSC, 128, 512], BF16)
    agout_d = nc.dram_tensor("agout", [NSC, NCORES, 128, 512], BF16, addr_space="Shared")
    bar_d = nc.dram_tensor("bar", [128], F32)
    barout_d = nc.dram_tensor("barout", [NCORES, 128], F32, addr_space="Shared")

    def sb(name, shape, dt):
        return nc.alloc_sbuf_tensor(name, shape, dt).ap()

    hsT = sb("hsT", [128, NHT, S], BF16)
    ident = sb("ident_sb", [128, 128], BF16)
    cosT2 = sb("cosT2", [128, S], BF16)
    ssinT2 = sb("ssinT2", [128, S], BF16)
    wq_sb = sb("wq_sb", [128, NHT, 128], BF16)
    wkk_sb = sb("wkk_sb", [128, NHT, 128], BF16)
    wv_sb = sb("wv_sb", [128, NHT, HD], BF16)
    wo_sb = sb("wo_sb", [128, NHT, 128], BF16)
    q2 = [sb(f"q2_{i}", [128, 512], BF16) for i in range(2)]
    qs2 = [sb(f"qs2_{i}", [128, 512], BF16) for i in range(2)]
    k2 = [sb(f"k2_{i}", [128, 512], BF16) for i in range(2)]
    ks2 = [sb(f"ks2_{i}", [128, 512], BF16) for i in range(2)]
    tmpa = sb("tmpa", [128, 512], BF16)
    tmpb = sb("tmpb", [128, 512], BF16)
    qT2 = sb("qT2", [128, S], BF16)
    kT2 = sb("kT2", [128, S], BF16)
    vT = sb("vT", [HD, S], BF16)
    vaug = sb("vaug", [128, NST, HD + 1], BF16)
    PT = [sb(f"PT{i}", [128, 1024], BF16) for i in range(NPT)]
    araw = [sb(f"araw{h}", [HD + 1, S], BF16) for h in range(2)]
    den64 = sb("den64", [64, 16], BF16)
    rcp64 = sb("rcp64", [64, 16], F32)
    rb = [[sb(f"rb{p}_{h}", [HD, 512], F32) for h in range(2)] for p in range(2)]
    lnd = sb("lnd", [1, 512], F32)
    rcp1 = [sb(f"rcp1_{h}", [1, 512], F32) for h in range(2)]
    ones1 = sb("ones1", [1, HD], F32)
    attn_sb = sb("attn_sb", [128, S], BF16)
    af = [sb(f"af{q}", [128, NHT, 512], BF16) for q in range(NSC)]
    out_ch = [sb(f"out_ch{i}", [128, 512], F32) for i in range(2)]
    dum_sb = sb("dum_sb", [1, 16], F32)
    dumb_sb = sb("dumb_sb", [1, 16], BF16)

    ps = nc.alloc_psum_tensor("psblob", [128, 4096], F32).ap()
    # front phase
    qp = [ps[:, 0:512], ps[:, 512:1024]]
    kp = [ps[:, 1024:1536], ps[:, 1536:2048]]
    vp = [ps[0:HD, 2048:2560], ps[0:HD, 2560:3072]]
    vtp = ps[:, 3072:3584].bitcast(BF16)          # [128, 1024]: 16 x [128,64]
    # attention: sp pair (g%2): banks (0,1) even g, (2,3) odd g
    spx = [ps[:, 0:1024], ps[:, 1024:2048]]       # exp reads FD=1024
    pv = [ps[0:HD + 1, 2048:2560], ps[0:HD + 1, 2560:3072]]  # per-head [65,512]
    opb = [ps[:, 3072:3584], ps[:, 3584:4096]]    # o_proj accum (parity)

    es = contextlib.ExitStack()
    SEM = lambda n: es.enter_context(nc.semaphore(n))
    sL = SEM("sL")        # SP weight/table loads (+16)
    sHSa = SEM("sHSa")    # hsT chunks 0,2 (SP)
    sHSb = SEM("sHSb")    # hsT chunks 1,3 (gpsimd)
    sQP = SEM("sQP"); sKP = SEM("sKP"); sVP = SEM("sVP")
    sQC = SEM("sQC"); sKC = SEM("sKC"); sVC = SEM("sVC")
    sSWQ = SEM("sSWQ")    # q swap DMAs (+16 each, 4/chunk, ACT queue)
    sSWK = SEM("sSWK")    # k swap DMAs (+16 each, 4/chunk, gpsimd queue)
    sQR = SEM("sQR"); sKR = SEM("sKR")
    sVA = SEM("sVA"); sMS = SEM("sMS")
    sSC = SEM("sSC")      # score MMs (2/g)
    sEX = SEM("sEX")      # exps (1/g)
    sPV = SEM("sPV")      # PV steps (1/g, on pv1 MM)
    sPVD = SEM("sPVD")    # pv0 stop MMs (1/quarter)
    sVTD = SEM("sVTD")    # PE drain after each chunk's v transposes
    sRW = SEM("sRW")      # raw attn copies out of psum (2/quarter)
    sDN64 = SEM("sDN64")  # den64 DMAs (+16 each, 3/quarter)
    sRC = SEM("sRC")      # recips (1/quarter)
    sDNS = SEM("sDNS")    # scr2 writes (+16 each, 2/quarter)
    sRB = SEM("sRB")      # rb broadcasts (+16 each, 3/quarter)
    sNM = SEM("sNM")      # normalize muls (2/quarter)
    sAG = SEM("sAG")      # agin DMAs (+16 each, 2/quarter)
    sCC = SEM("sCC")      # collectives (4 AG + barrier)
    sAF = SEM("sAF")      # af loads on SP queue (+16 each, 1/quarter + dummy)
    sOP = SEM("sOP")      # o_proj stop MMs (1/quarter)
    sOC = SEM("sOC"); sOD = SEM("sOD")
    sRC3 = SEM("sRC3")    # ACT ln/exp recips for quarter 3 (2)
    sBC3 = SEM("sBC3")    # PE outer-product broadcasts for quarter 3 (1)

    with nc.Block() as block:

        # ===== SP: loads + den chain + af + stores =====
        @block.sync
        def _(sync):
            sync.dma_start(out=wkk_sb, in_=wkk_d[:]).then_inc(sL, 16)
            # chunk 0 in two halves so the k projection starts sooner
            sync.dma_start(out=hsT[:, 0:4, 0:512], in_=hsT_d[0][:, 0:2048]).then_inc(sHSa, 16)
            sync.dma_start(out=hsT[:, 4:8, 0:512], in_=hsT_d[0][:, 2048:4096]).then_inc(sHSa, 16)
            sync.dma_start(out=wq_sb, in_=wq_d[:]).then_inc(sL, 16)
            sync.dma_start(out=wv_sb, in_=wv_d[:]).then_inc(sL, 16)
            sync.dma_start(out=hsT[:, :, 1024:1536], in_=hsT_d[2]).then_inc(sHSa, 16)
            sync.dma_start(out=ident, in_=id_d[:]).then_inc(sL, 16)
            sync.dma_start(out=cosT2, in_=cosT_d[:]).then_inc(sL, 16)
            sync.dma_start(out=ssinT2, in_=ssinT_d[:]).then_inc(sL, 16)
            sync.dma_start(out=wo_sb, in_=wo_d[:]).then_inc(sL, 16)

            def den_agin(qc):
                sl = slice(qc * 512, (qc + 1) * 512)
                # rcp -> DRAM; read-back completion proves DRAM visibility
                # for the gpsimd queue's broadcast reads
                sync.wait_ge(sRC, qc + 1)
                sync.dma_start(out=scr2_d[qc], in_=rcp64).then_inc(sDNS, 16)
                sync.dma_start(out=dum_sb, in_=scr2_d[qc, 0:16]).then_inc(sDNS, 16)
                sync.wait_ge(sNM, 2 * qc + 2)
                sync.dma_start(out=agin_d[qc], in_=attn_sb[:, sl]).then_inc(sAG, 16)
                sync.dma_start(out=dumb_sb, in_=agin_d[qc][0:1, 0:16]).then_inc(sAG, 16)

            def af_load(qc):
                # A collective's completion sem does NOT order remote ranks'
                # inbound pushes against our reads. Per-sender SDMA queues are
                # FIFO, so entering the NEXT collective proves every rank's
                # previous pushes landed: gate af[qc] on collective qc+1
                # (the trailing barrier gather for qc=3). sCC counts the
                # startup warmup gather first.
                sync.wait_ge(sCC, qc + 3)
                # af[p, j, q] = agout[core j, row p, q]; slab j of Wo is rows
                # j*128..j*128+127 (natural order: row (2c+h)*64+d = c*128+p)
                sync.dma_start(
                    out=af[qc],
                    in_=agout_d[qc].rearrange("c p q -> p c q"),
                ).then_inc(sAF, 16)

            def store(qc):
                sync.wait_ge(sOC, qc + 1)
                sync.dma_start(
                    out=out_d[:, qc * 512:(qc + 1) * 512], in_=out_ch[qc % 2]
                ).then_inc(sOD, 16)

            # all den/agin chains issued before any af load so collective
            # completions never gate a later quarter's den chain
            den_agin(0)
            den_agin(1)
            den_agin(2)
            den_agin(3)
            af_load(0)
            af_load(1)
            af_load(2)
            # +1-shift proof DMAs: a later DMA on this FIFO queue proves the
            # preceding af load's SBUF writes are visible to the PE
            sync.dma_start(out=dum_sb, in_=scr2_d[0, 0:16]).then_inc(sAF, 16)
            store(0)
            af_load(3)
            sync.dma_start(out=dum_sb, in_=scr2_d[0, 0:16]).then_inc(sAF, 16)
            store(1)
            store(2)
            store(3)
            sync.wait_ge(sOD, 16 * NSC)

        # ================= PE =================
        @block.tensor
        def _(tensor):
            for sc in range(NSC):
                if sc == 2:
                    tensor.wait_ge(sHSa, 48)
                elif sc % 2 == 1:
                    tensor.wait_ge(sHSb, 16 * (sc // 2 + 1))
                sl = slice(sc * 512, (sc + 1) * 512)
                # k projection (duplicated into both halves)
                if sc == 0:
                    tensor.wait_ge(sL, 16)
                if sc >= 2:
                    tensor.wait_ge(sKC, sc - 1)
                for ht in range(NHT):
                    # chunk 0 arrives in two halves on the vector queue
                    if sc == 0 and ht == 0:
                        tensor.wait_ge(sHSa, 16)
                    if sc == 0 and ht == 4:
                        tensor.wait_ge(sHSa, 32)
                    inst = tensor.matmul(
                        kp[sc % 2], wkk_sb[:, ht, :], hsT[:, ht, sl],
                        start=(ht == 0), stop=(ht == NHT - 1),
                    )
                inst.then_inc(sKP, 1)
                # q projection (2 heads packed)
                if sc == 0:
                    tensor.wait_ge(sL, 32)
                if sc >= 2:
                    tensor.wait_ge(sQC, sc - 1)
                for ht in range(NHT):
                    inst = tensor.matmul(
                        qp[sc % 2], wq_sb[:, ht, :], hsT[:, ht, sl],
                        start=(ht == 0), stop=(ht == NHT - 1),
                    )
                inst.then_inc(sQP, 1)
                # v projection
                if sc == 0:
                    tensor.wait_ge(sL, 48)
                if sc >= 2:
                    tensor.wait_ge(sVC, sc - 1)
                for ht in range(NHT):
                    inst = tensor.matmul(
                        vp[sc % 2], wv_sb[:, ht, :], hsT[:, ht, sl],
                        start=(ht == 0), stop=(ht == NHT - 1),
                    )
                inst.then_inc(sVP, 1)
                # v transposes for this chunk's 4 ktiles
                if sc == 0:
                    tensor.wait_ge(sL, 64)
                tensor.wait_ge(sVC, sc + 1)
                for j in range(4):
                    kt = 4 * sc + j
                    inst = tensor.transpose(
                        vtp[:, kt * 64:(kt + 1) * 64],
                        vT[:, kt * 128:(kt + 1) * 128],
                        ident[0:HD, 0:HD],
                    )
                inst.then_inc(sVTD, 1)

            # ================= attention (4 query quarters) =============
            tensor.wait_ge(sQC, NSC)
            tensor.wait_ge(sKC, NSC)
            tensor.wait_ge(sVC, NSC)
            tensor.wait_ge(sMS, 1)

            def pv_step(gp):
                qcp, ktp = gp // NST, gp % NST
                tensor.wait_ge(sEX, gp + 1)
                tensor.wait_ge(sVA, ktp + 1)
                if ktp == 0 and qcp > 0:
                    tensor.wait_ge(sRW, 2 * qcp)
                st, sp_ = (ktp == 0), (ktp == NST - 1)
                i0 = tensor.matmul(
                    pv[0], vaug[:, ktp, :], PT[gp % NPT][:, 0:512],
                    start=st, stop=sp_,
                )
                i1 = tensor.matmul(
                    pv[1], vaug[:, ktp, :], PT[gp % NPT][:, 512:1024],
                    start=st, stop=sp_,
                )
                i1.then_inc(sPV, 1)
                if sp_:
                    i0.then_inc(sPVD, 1)

            for g in range(NG):
                qc, kt = g // NST, g % NST
                if g >= 2:
                    tensor.wait_ge(sEX, g - 1)   # sp pair free
                if kt == 0:
                    tensor.wait_ge(sQR, qc + 1)
                if qc == 0:
                    tensor.wait_ge(sKR, kt // 4 + 1)
                qsl = slice(qc * 512, (qc + 1) * 512)
                for h in range(2):
                    hp = slice(h * 64, (h + 1) * 64)
                    tensor.matmul(
                        ps[:, (g % 2) * 1024 + h * 512:(g % 2) * 1024 + (h + 1) * 512],
                        kT2[hp, kt * 128:(kt + 1) * 128],
                        qT2[hp, qsl],
                        start=True, stop=True,
                    ).then_inc(sSC, 1)
                if g >= 1:
                    pv_step(g - 1)
            pv_step(NG - 1)

            # ================= o_proj (per quarter, 8-MM accum) =========
            tensor.wait_ge(sL, 112)
            # af gating +1-shifted; sAF increments: af0=16, af1=32, af2=48,
            # dummy=64, af3=80, dummy=96
            for qc, afw in enumerate((32, 48, 64, 96)):
                tensor.wait_ge(sAF, afw)
                if qc >= 2:
                    tensor.wait_ge(sOC, qc - 1)
                for j in range(NHT):
                    inst = tensor.matmul(
                        opb[qc % 2], wo_sb[:, j, :], af[qc][:, j, :],
                        start=(j == 0), stop=(j == NHT - 1),
                    )
                inst.then_inc(sOP, 1)

        # ================= DVE =================
        @block.vector
        def _(vector):
            for sc in range(NSC):
                sl = slice(sc * 512, (sc + 1) * 512)
                vector.wait_ge(sQP, sc + 1)
                if sc >= 2:
                    vector.wait_ge(sSWQ, 64 * (sc - 1))  # q2 swap reads done
                vector.tensor_copy(q2[sc % 2], qp[sc % 2]).then_inc(sQC, 1)
                vector.wait_ge(sKP, sc + 1)
                if sc >= 2:
                    vector.wait_ge(sSWK, 64 * (sc - 1))  # k2 swap reads done
                vector.tensor_copy(k2[sc % 2], kp[sc % 2]).then_inc(sKC, 1)
                vector.wait_ge(sVP, sc + 1)
                vector.tensor_copy(vT[:, sl], vp[sc % 2]).then_inc(sVC, 1)
                # rope q (both heads packed)
                if sc == 0:
                    vector.wait_ge(sL, 96)
                vector.wait_ge(sSWQ, 64 * (sc + 1))
                vector.tensor_mul(tmpa, q2[sc % 2], cosT2[:, sl])
                vector.tensor_mul(tmpb, qs2[sc % 2], ssinT2[:, sl])
                vector.tensor_add(qT2[:, sl], tmpa, tmpb).then_inc(sQR, 1)
                # rope k
                vector.wait_ge(sSWK, 64 * (sc + 1))
                vector.tensor_mul(tmpa, k2[sc % 2], cosT2[:, sl])
                vector.tensor_mul(tmpb, ks2[sc % 2], ssinT2[:, sl])
                vector.tensor_add(kT2[:, sl], tmpa, tmpb).then_inc(sKR, 1)
                # vaug copies
                vector.wait_ge(sVTD, sc + 1)
                for j in range(4):
                    kt = 4 * sc + j
                    vector.tensor_copy(vaug[:, kt, 0:HD], vtp[:, kt * 64:(kt + 1) * 64]).then_inc(sVA, 1)

            # per-quarter: raw copies (release pv psum), recip, normalize
            for qc in range(NSC):
                sl = slice(qc * 512, (qc + 1) * 512)
                vector.wait_ge(sPVD, qc + 1)
                vector.wait_ge(sPV, NST * (qc + 1))
                vector.tensor_copy(araw[0][:, sl], pv[0]).then_inc(sRW, 1)
                vector.tensor_copy(araw[1][:, sl], pv[1]).then_inc(sRW, 1)
                vector.wait_ge(sDN64, 48 * (qc + 1))
                if qc >= 1:
                    vector.wait_ge(sDNS, 32 * qc - 16)  # rcp64 drained (qc-1)
                vector.reciprocal(rcp64, den64).then_inc(sRC, 1)
                # +1-DMA shift: sw-DGE completion sems can fire before the
                # data is visible; the NEXT DMA's completion (same queue,
                # FIFO) proves this one's writes landed
                vector.wait_ge(sRB, 48 * (qc + 1))
                vector.tensor_mul(attn_sb[0:64, sl], araw[0][0:HD, sl], rb[qc % 2][0]).then_inc(sNM, 1)
                vector.tensor_mul(attn_sb[64:128, sl], araw[1][0:HD, sl], rb[qc % 2][1]).then_inc(sNM, 1)
            # out copies (after the last quarter's den chain so the AG3
            # critical path is not blocked behind o_proj results)
            for oq in range(NSC):
                vector.wait_ge(sOP, oq + 1)
                if oq >= 2:
                    vector.wait_ge(sOD, 16 * (oq - 1))
                vector.tensor_copy(out_ch[oq % 2], opb[oq % 2]).then_inc(sOC, 1)

        # ====== ACT: hsT ch1/ch3 loads + swap DMAs + exp (hw DGE) ======
        @block.scalar
        def _(scalar):
            scalar.dma_start(out=hsT[:, :, 512:1024], in_=hsT_d[1]).then_inc(sHSb, 16)
            scalar.dma_start(out=hsT[:, :, 1536:2048], in_=hsT_d[3]).then_inc(sHSb, 16)
            for sc in range(NSC):
                scalar.wait_ge(sQC, sc + 1)
                if sc >= 2:
                    scalar.wait_ge(sQR, sc - 1)  # qs2 buf consumed
                for b in range(2):
                    scalar.dma_start(
                        out=qs2[sc % 2][b * 64:b * 64 + 32, :],
                        in_=q2[sc % 2][b * 64 + 32:b * 64 + 64, :],
                    ).then_inc(sSWQ, 16)
                    scalar.dma_start(
                        out=qs2[sc % 2][b * 64 + 32:b * 64 + 64, :],
                        in_=q2[sc % 2][b * 64:b * 64 + 32, :],
                    ).then_inc(sSWQ, 16)
                scalar.wait_ge(sKC, sc + 1)
                if sc >= 2:
                    scalar.wait_ge(sKR, sc - 1)  # ks2 buf consumed
                for b in range(2):
                    scalar.dma_start(
                        out=ks2[sc % 2][b * 64:b * 64 + 32, :],
                        in_=k2[sc % 2][b * 64 + 32:b * 64 + 64, :],
                    ).then_inc(sSWK, 16)
                    scalar.dma_start(
                        out=ks2[sc % 2][b * 64 + 32:b * 64 + 64, :],
                        in_=k2[sc % 2][b * 64:b * 64 + 32, :],
                    ).then_inc(sSWK, 16)
            # exps: one per global step covers both heads (FD=1024)
            for g in range(NG):
                scalar.wait_ge(sSC, 2 * g + 2)
                if g >= NPT:
                    scalar.wait_ge(sPV, g - (NPT - 1))  # PT slot consumed
                scalar.activation(
                    PT[g % NPT][:, :], spx[g % 2], EXP, scale=0.125,
                ).then_inc(sEX, 1)

        # ==== GPSIMD: memset, sw-DGE den/broadcast DMAs, collectives ====
        @block.gpsimd
        def _(gpsimd):
            gpsimd.memset(vaug[:, :, HD:HD + 1], 1.0).then_inc(sMS, 1)
            gpsimd.memset(ones1, 1.0).then_inc(sMS, 1)
            # warmup gather issued immediately: absorbs cross-core launch
            # skew + CC-engine startup barrier under the compute phase, so
            # the data gathers later run in lockstep
            gpsimd.collective_compute(
                "AllGather",
                mybir.AluOpType.bypass,
                replica_groups=[list(range(NCORES))],
                ins=[bar_d[:]],
                outs=[barout_d[:]],
            ).then_inc(sCC, 1)
            for qc in range(NSC):
                sl = slice(qc * 512, (qc + 1) * 512)
                gpsimd.wait_ge(sRW, 2 * qc + 2)
                if qc >= 1:
                    gpsimd.wait_ge(sRC, qc)  # den64 consumed by recip qc-1
                gpsimd.dma_start(
                    out=den64[0:32, :], in_=araw[0][HD:HD + 1, sl],
                ).then_inc(sDN64, 16)
                gpsimd.dma_start(
                    out=den64[32:64, :], in_=araw[1][HD:HD + 1, sl],
                ).then_inc(sDN64, 16)
                # dummy follow-up DMA: its completion proves den64 is
                # visible (sw-DGE sems can fire before the data lands)
                gpsimd.dma_start(
                    out=dumb_sb, in_=araw[0][HD:HD + 1, qc * 512:qc * 512 + 16],
                ).then_inc(sDN64, 16)
                gpsimd.wait_ge(sDNS, 32 * (qc + 1))
                if qc >= 2:
                    gpsimd.wait_ge(sNM, 2 * qc - 2)  # rb parity buf consumed
                for h in range(2):
                    gpsimd.dma_start(
                        out=rb[qc % 2][h],
                        in_=bass.AP(scr2_d[:].tensor, qc * 1024 + h * 512, [[0, HD], [1, 512]]),
                    ).then_inc(sRB, 16)
                gpsimd.dma_start(
                    out=dum_sb, in_=bass.AP(scr2_d[:].tensor, qc * 1024, [[0, 1], [1, 16]]),
                ).then_inc(sRB, 16)
                gpsimd.wait_ge(sAG, 32 * (qc + 1))
                gpsimd.collective_compute(
                    "AllGather",
                    mybir.AluOpType.bypass,
                    replica_groups=[list(range(NCORES))],
                    ins=[agin_d[qc]],
                    outs=[agout_d[qc]],
                ).then_inc(sCC, 1)
            # barrier collective: its completion proves every rank's AG3
            # pushes into our agout landed (FIFO per sender SDMA queue)
            gpsimd.collective_compute(
                "AllGather",
                mybir.AluOpType.bypass,
                replica_groups=[list(range(NCORES))],
                ins=[bar_d[:]],
                outs=[barout_d[:]],
            ).then_inc(sCC, 1)

    es.close()
    return nc


_NC_CACHE = None


def kernel(hidden_states, cos, sin, attention_mask, Wq, Wk, Wv, Wo):
    global _NC_CACHE
    if _NC_CACHE is None:
        _NC_CACHE = build_kernel()
    nc = _NC_CACHE
    hs2 = np.asarray(hidden_states, dtype=np.float32).reshape(S, HID)
    # hsT chunk-contiguous: [sc, p, t*512] with row (t*128+p) of hs.T
    hsT = np.ascontiguousarray(hs2.T.astype(NPBF))                    # [HID, S]
    hsT_c = np.ascontiguousarray(
        hsT.reshape(NHT, 128, NSC, 512).transpose(2, 1, 0, 3).reshape(NSC, 128, NHT * 512))
    cosT = np.asarray(cos, dtype=np.float32).reshape(S, HD).T         # [64, S]
    sinT = np.asarray(sin, dtype=np.float32).reshape(S, HD).T
    ssinT = sinT.copy()
    ssinT[0:32, :] *= -1.0
    cosT2 = np.ascontiguousarray(np.concatenate([cosT, cosT], 0).astype(NPBF))
    ssinT2 = np.ascontiguousarray(np.concatenate([ssinT, ssinT], 0).astype(NPBF))
    Wq = np.asarray(Wq, dtype=np.float32)
    Wk = np.asarray(Wk, dtype=np.float32)
    Wv = np.asarray(Wv, dtype=np.float32)
    Wo = np.asarray(Wo, dtype=np.float32)
    ident = np.eye(128, dtype=np.float32).astype(NPBF)

    def warr(w):  # [1024, X] -> [128, 8*X] partition-major contiguous
        x = w.shape[1]
        return np.ascontiguousarray(
            w.reshape(NHT, 128, x).transpose(1, 0, 2).reshape(128, NHT * x).astype(NPBF))

    in_maps = []
    for c in range(NCORES):
        g = c // 2
        wk_g = Wk[:, g * HD:(g + 1) * HD]
        in_maps.append({
            "hst": hsT_c,
            "wq": warr(Wq[:, c * 128:(c + 1) * 128]),
            "wkk": warr(np.concatenate([wk_g, wk_g], axis=1)),
            "wv": warr(Wv[:, g * HD:(g + 1) * HD]),
            "wo": warr(Wo[:, c * 128:(c + 1) * 128]),
            "cost": cosT2,
            "ssint": ssinT2,
            "ident": ident,
        })
    res = run_bass_kernel_spmd(nc, in_maps, core_ids=list(range(NCORES)),
                               trace=bool(int(os.environ.get("KERNEL_TRACE", "0"))))
    out = np.empty((S, HID), dtype=np.float32)
    for c in range(NCORES):
        out[:, c * 128:(c + 1) * 128] = res.results[c]["out_t"].T
    kernel.last_results = res
    return out.reshape(1, S, HID)


if __name__ == "__main__":
    import tempfile
    from concourse.bass_utils import compile_bass_kernel
    nc = build_kernel()
    with tempfile.TemporaryDirectory() as td:
        compile_bass_kernel(nc, td)
    print("COMPILE OK")
